# revision 1
# baseline (speedup 1.0000x reference)
"""Trainium2 Bass kernel for nn_Conv2d_86191403696259 (fp16 bands layout).

Originally: 1x HBM input read. Layout p = 32*dh + 3j + ic (K=88, zero-weight gaps).

Per chunk: DMA-load staging [24, (S+2)*WP] once; DVE-copy the three dh-shifted
views into 32-aligned partition groups of the slab; matmul as in v1.
PSUM->SBUF copies on ScalarE; per-s output DMAs.
"""

import ml_dtypes
import numpy as np

import concourse.bass as bass
import concourse.mybir as mybir
import concourse.tile as tile
from concourse import bacc
from concourse.bass_utils import run_bass_kernel_spmd

IC, OC, KH, KW = 3, 16, 3, 3
H = W = 2048
N_CORES = 8
RPC = H // N_CORES          # 256
HP = RPC + 2                # 258
WP = W + 2                  # 2050

NB = 8                      # bands
BR = RPC // NB              # 32 rows per band
S = 4                       # s-steps per chunk
NCHUNK = BR // S            # 8
NWT = W // 512              # 4
KP = 88                     # contraction partitions (with gaps)

F32 = mybir.dt.float32
F32R = mybir.dt.float32r
BF16 = mybir.dt.bfloat16
FP16 = mybir.dt.float16
DT = FP16


def build_nc() -> bass.Bass:
    nc = bacc.Bacc("TRN2", target_bir_lowering=False, debug=False)
    x = nc.dram_tensor("x", [IC, HP, WP], DT, kind="ExternalInput")
    wd = nc.dram_tensor("wd", [KW, KP, 128], DT, kind="ExternalInput")
    out = nc.dram_tensor("out", [OC, RPC, W], F32, kind="ExternalOutput")

    with tile.TileContext(nc) as tc:
        with (
            tc.tile_pool(name="wpool", bufs=1) as wpool,
            tc.tile_pool(name="slabp", bufs=1) as slab_pool,
            tc.tile_pool(name="stgin", bufs=2) as stgin_pool,
            tc.tile_pool(name="stgout", bufs=3) as stgout_pool,
            tc.tile_pool(name="psum", bufs=2, space="PSUM") as psum_pool,
        ):
            w_sb = wpool.tile([KP, KW * 128], DT)
            nc.sync.dma_start(out=w_sb[:, :], in_=wd.rearrange("dw p m -> p dw m"))

            # one persistent slab, two halves (chunk parity); zero the gap
            # partition groups once so matmul contraction reads finite zeros.
            slab = slab_pool.tile([KP, 2 * S * WP], DT)
            nc.vector.memset(slab[:, :], 0.0)

            for kc in range(NCHUNK):
                half = (kc % 2) * S * WP
                stin = stgin_pool.tile([24, (S + 2) * WP], DT, tag="stin")
                for u in range(S + 2):
                    rs = S * kc + u
                    src = x[:, rs : rs + (NB - 1) * BR + 1 : BR, :]
                    nc.sync.dma_start(
                        out=stin[:, u * WP : (u + 1) * WP],
                        in_=src.rearrange("ic j w -> j ic w"),
                    )
                for dh in range(KH):
                    nc.vector.tensor_copy(
                        out=slab[32 * dh : 32 * dh + 24, half : half + S * WP],
                        in_=stin[:, dh * WP : dh * WP + S * WP],
                    )

                for s in range(S):
                    ps = psum_pool.tile([128, W], F32, tag="ps")
                    for dw in range(KW):
                        for wt in range(NWT):
                            nc.tensor.matmul(
                                out=ps[:, wt * 512 : (wt + 1) * 512],
                                lhsT=w_sb[:, dw * 128 : (dw + 1) * 128],
                                rhs=slab[
                                    :,
                                    half + s * WP + wt * 512 + dw : half
                                    + s * WP
                                    + wt * 512
                                    + dw
                                    + 512,
                                ],
                                start=(dw == 0),
                                stop=(dw == KW - 1),
                            )
                    stg = stgout_pool.tile([128, W], F32, tag="stg")
                    nc.scalar.copy(out=stg[:, :], in_=ps[:, :])
                    rs = S * kc + s
                    dst = out[:, rs : rs + (NB - 1) * BR + 1 : BR, :]
                    # issue stores from ScalarE (HWDGE) so the Sync engine's
                    # FIFO only carries input loads and never blocks them
                    # behind store->copy->matmul dependency chains.
                    nc.scalar.dma_start(
                        out=dst.rearrange("oc j w -> j oc w"), in_=stg[:, :]
                    )

    nc.compile()
    return nc


def make_wdiag(kernel: np.ndarray) -> np.ndarray:
    """kernel [OC, IC, KH, KW] -> lhsT stack [KW, KP, 128], gaps zeroed."""
    wdg = np.zeros((KW, KP, 128), np.float32)
    for dw in range(KW):
        for dh in range(KH):
            for j in range(NB):
                for ic in range(IC):
                    wdg[dw, 32 * dh + 3 * j + ic, 16 * j : 16 * j + OC] = kernel[
                        :, ic, dh, dw
                    ]
    return wdg


_NC_CACHE = {}


def kernel(x: np.ndarray, kernel: np.ndarray) -> np.ndarray:
    assert x.shape == (IC, H, W) and kernel.shape == (OC, IC, KH, KW)
    x = np.ascontiguousarray(x, np.float32)
    kernel = np.ascontiguousarray(kernel, np.float32)

    if "nc" not in _NC_CACHE:
        _NC_CACHE["nc"] = build_nc()
    nc = _NC_CACHE["nc"]

    x_pad = np.zeros((IC, H + 2, W + 2), np.float16)
    x_pad[:, 1:-1, 1:-1] = x.astype(np.float16)
    wd = make_wdiag(kernel).astype(np.float16)

    in_maps = []
    for c in range(N_CORES):
        slab = np.ascontiguousarray(x_pad[:, c * RPC : c * RPC + HP, :])
        in_maps.append({"x": slab, "wd": wd})

    res = run_bass_kernel_spmd(nc, in_maps, core_ids=list(range(N_CORES)))
    outs = [res.results[c]["out"] for c in range(N_CORES)]
    return np.concatenate(outs, axis=1)

